# revision 27
# baseline (speedup 1.0000x reference)
"""Trainium2 Bass kernel for DGLBatchCapsuleLayer (capsule dynamic routing).

Math (reference):
    u_hat[c,j,b,t] = sum_i W[c,j,t,i] * x[b,i,c]
    3 routing iterations:
        c_ij = softmax_j(b_ij)
        s[j,b,t] = sum_c c_ij[c,j] * u_hat[c,j,b,t]
        v = squash_t(s)
        b_ij += mean_b <v_j, u_hat_cj> (skipped on last iter - unused)
    out = v as [B, J, S, 1]

u_hat (360 MB) is never materialized; both routing contractions are
matmuls against x with k=(c,i):
    s[b,(j,t)]  = sum_k x[b,k] * (c*W)[k,(j,t)]
    M[k,(j,t)]  = sum_b x[b,k] * v[b,(j,t)]
    bdelta[c,j] = sum_{t,i} W[k,(j,t)] * M[k,(j,t)]
b_ij lives i-replicated in (c,i) rows; the i-reduction + replication of
bdelta is one matmul against a block-diagonal 8x8-ones matrix.

Distribution: iterations 1-2 run fully REPLICATED on all 8 cores;
iteration 3 computes only each core's B/8-row output shard. Cross-core
paths were measured and rejected: the first collective_compute of an
execution costs ~70us (channel setup, unoverlappable with compute) and
a gpsimd remote_dma all-exchange still pays ~60us of launch
skew/rendezvous at the first sync point, so an 8-way C-sharded variant
(~45us of compute) would land no better than ~110us and adds
device-hang risk.

Schedule highlights (182us -> ~154us):
  - b_ij updates in four k-groups [24,24,16,8]; each group's update ->
    softmax (no max-subtract: |b_ij| < ~1 so exp is safe) -> t-broadcast
    -> c*W chain runs under the M-phase as its tja slice completes, and
    the PE consumes the next iteration's s-matmul k-chunks in group
    order, so iteration boundaries cost the PE little.
  - W*M drain is split across engines per 3-chunk PSUM group: vector
    mult+reduce for one group per piece, scalar PSUM->SBUF copy +
    gpsimd multiply + vector reduce for the rest; piece 8 stays fully
    on vector to cut latency into the last chain. tja accumulates in
    bf16 (error ~1e-3 on b_ij - negligible vs the softmax scale).
  - The two big groups' t-broadcasts are materialized on the scalar
    engine (vector multiply then runs at the 16-bit 2x rate); the two
    small trailing groups use a fused stride-0 broadcast multiply for
    minimal serial latency into iteration 3.
  - Loads ride separate per-engine DMA queues, earliest-needed first,
    with the first xtf piece and wtf group split so the first matmul
    issues at ~10us; xtb (iteration-3 shard operand) loads after the
    iteration-0 s-phase to keep head bandwidth for xtf; x2f re-streams
    per M-phase through a 4-deep rotating buffer (SBUF cannot hold
    both x layouts).
  - A dummy scalar sqrt after the last softmax exp prefetches the
    exp->sqrt activation-table swap off the squash critical path.
"""

import numpy as np
import ml_dtypes

NCORES = 8
B, I, C, J, S = 512, 8, 1152, 10, 16
BSH = B // NCORES          # 64-row output shard per core
KF = C * I                 # 9216 full contraction length, k = c*8+i
NKF = KF // 128            # 72 k-chunks
NQ = 9                     # streamed pieces (8 chunks each)
NB = B // 128              # 4 batch chunks
JT = J * S                 # 160
NUM_ROUTING = 3
# b_ij update groups by k-chunk ranges (pieces [0-2],[3-5],[6-7],[8]):
# the trailing groups are small so their softmax/bcast/wc chains finish
# right after the M-phase instead of ~8us later
GRP = [(0, 24), (24, 48), (48, 64), (64, 72)]

_BF16 = ml_dtypes.bfloat16

_built = None


def _build():
    import concourse.bass as bass  # noqa: F401
    import concourse.bacc as bacc
    import concourse.mybir as mybir
    import concourse.tile as tile

    f32 = mybir.dt.float32
    f16 = mybir.dt.float16
    bf16 = mybir.dt.bfloat16
    mult = mybir.AluOpType.mult
    add = mybir.AluOpType.add
    AX = mybir.AxisListType.X
    Exp = mybir.ActivationFunctionType.Exp

    nc = bacc.Bacc(
        "TRN2",
        target_bir_lowering=False,
        debug=False,
        num_devices=NCORES,
    )

    # all inputs host-majorized to [partition, ...] with contiguous rows
    xtf_d = nc.dram_tensor("xtf", [128, NQ, 8, B], bf16,
                           kind="ExternalInput")
    wtf_d = nc.dram_tensor("wtf", [128, NKF, J, S], bf16,
                           kind="ExternalInput")
    x2f_d = nc.dram_tensor("x2f", [128, NQ, NB, 8 * 128], bf16,
                           kind="ExternalInput")
    xtb_d = nc.dram_tensor("xtb", [128, NKF, BSH], bf16,
                           kind="ExternalInput")
    out_d = nc.dram_tensor("out", [BSH, JT], f32, kind="ExternalOutput")

    # block-diag 8x8 ones: reduces over i and replicates, in one matmul
    rep_np = np.kron(np.eye(16), np.ones((8, 8))).astype(ml_dtypes.bfloat16)
    rep_d = nc.inline_tensor(rep_np, name="repind")

    with tile.TileContext(nc) as tc:
        with (
            nc.allow_low_precision(reason="bf16 tja/squash accumulation"),
            tc.tile_pool(name="pers", bufs=1) as pers,
            tc.tile_pool(name="work", bufs=2) as work,
            tc.tile_pool(name="x2rot", bufs=4) as x2rot,
            tc.tile_pool(name="tmprot", bufs=6) as tmprot,
            tc.tile_pool(name="cold", bufs=2) as cold,
            tc.tile_pool(name="spsum", bufs=1, space="PSUM") as spsum,
            tc.tile_pool(name="mpsum", bufs=3, space="PSUM") as mpsum,
            tc.tile_pool(name="bpsum", bufs=1, space="PSUM") as bpsum,
        ):
            # ---- persistent SBUF ----
            xtf_t = [
                pers.tile([128, 8, B], bf16, name=f"xtf{q}")
                for q in range(NQ)
            ]
            wtf_sb = pers.tile([128, NKF, J, S], bf16, name="wtf_sb")
            xtb_sb = pers.tile([128, NKF, BSH], bf16, name="xtb_sb")
            rep_sb = pers.tile([128, 128], bf16, name="rep_sb")
            b_rep = pers.tile([128, NKF, J], f32, name="b_rep")
            wc = pers.tile([128, NKF, J, S], bf16, name="wc")
            tja = pers.tile([128, NKF, J], bf16, name="tja")
            s16 = pers.tile([128, NB, J, S], f16, name="s16")
            v_bf = pers.tile([128, NB, JT], bf16, name="v_bf")
            vout = pers.tile([BSH, JT], f32, name="vout")

            # loads on separate queues: first-needed first, and the
            # first pieces split so the first matmul starts sooner
            nc.sync.dma_start(xtf_t[0][:, :2], xtf_d.ap()[:, 0, :2])
            nc.sync.dma_start(xtf_t[0][:, 2:], xtf_d.ap()[:, 0, 2:])
            for q in range(1, NQ):
                nc.sync.dma_start(xtf_t[q][:], xtf_d.ap()[:, q])
            nc.scalar.dma_start(wtf_sb[:, 0:4], wtf_d.ap()[:, 0:4])
            nc.scalar.dma_start(wtf_sb[:, 4:24], wtf_d.ap()[:, 4:24])
            for lo, hi in GRP[1:]:
                nc.scalar.dma_start(
                    wtf_sb[:, lo:hi], wtf_d.ap()[:, lo:hi])
            nc.gpsimd.dma_start(rep_sb[:], rep_d.ap())
            nc.vector.memset(b_rep[:], 0.0)

            sps = [
                spsum.tile([128, JT], f32, name=f"sp{g}", tag=f"sp{g}")
                for g in range(NB)
            ]

            def squash_full():
                # v_bf[b,(j,t)] = squash_t(s), s in the 4 sps psum banks
                for g in range(NB):
                    nc.vector.tensor_copy(s16[:, g], sps[g][:])
                sq = work.tile([128, NB, J, S], f16, name="sq", tag="sq")
                nc.vector.tensor_mul(sq[:], s16[:], s16[:])
                msq = work.tile([128, NB, J], f16, name="msq", tag="msq")
                nc.vector.reduce_sum(msq[:], sq[:], axis=AX)
                d1 = work.tile([128, NB, J], f32, name="d1", tag="d1")
                nc.vector.tensor_scalar_add(d1[:], msq[:], 1.0)
                rd1 = work.tile([128, NB, J], f32, name="rd1", tag="rd1")
                nc.vector.reciprocal(rd1[:], d1[:])
                mag = work.tile([128, NB, J], f32, name="mag", tag="mag")
                nc.scalar.sqrt(mag[:], msq[:])
                f2 = work.tile([128, NB, J], f32, name="f2", tag="f2")
                nc.vector.tensor_mul(f2[:], mag[:], rd1[:])
                nc.vector.tensor_mul(
                    v_bf[:].rearrange("p g (j t) -> p g j t", t=S),
                    s16[:],
                    f2[:].unsqueeze(3).broadcast_to([128, NB, J, S]),
                )

            def chain_group(lo, hi):
                # moderate priority: beat same-window M-drain ops in the
                # scheduler's ordering without starving the drains
                with tc.high_priority(offset=40):
                    _chain_group(lo, hi)

            def _chain_group(lo, hi):
                # b_rep[g] += bdelta; softmax_j; t-broadcast; wc[g] = c*W
                gk = hi - lo
                bd = bpsum.tile([128, gk * J], f32, name="bd", tag="bd")
                nc.tensor.matmul(
                    bd[:], rep_sb[:],
                    tja[:, lo:hi].rearrange("p k j -> p (k j)"),
                    start=True, stop=True,
                )
                br = b_rep[:, lo:hi]
                # b_rep += bd * 10/B  (x10 undoes the host-side W*0.1)
                nc.vector.scalar_tensor_tensor(
                    br.rearrange("p k j -> p (k j)"),
                    bd[:], 10.0 / B,
                    br.rearrange("p k j -> p (k j)"),
                    op0=mult, op1=add,
                )
                # softmax over j without max-subtraction (|b| < ~1)
                ex = work.tile([128, gk, J], f32, name="ex", tag="ex")
                nc.scalar.activation(ex[:], br, Exp)
                den = work.tile([128, gk], f32, name="den", tag="den")
                nc.vector.reduce_sum(den[:], ex[:], axis=AX)
                rden = work.tile([128, gk], f32, name="rden", tag="rden")
                nc.vector.reciprocal(rden[:], den[:])
                cb = work.tile([128, gk, J], bf16, name="cb", tag="cb")
                # cb = (ex * 10) * (1/den)
                nc.vector.scalar_tensor_tensor(
                    cb[:], ex[:], 10.0,
                    rden[:].unsqueeze(2).broadcast_to([128, gk, J]),
                    op0=mult, op1=mult,
                )
                if gk >= 24:
                    # big group: materialize the t-broadcast on scalar so
                    # the vector multiply runs at the 16-bit 2x rate
                    ce = work.tile([128, gk, J, S], bf16, name="ce",
                                   tag="ce")
                    nc.scalar.copy(
                        ce[:],
                        cb[:].unsqueeze(3).broadcast_to([128, gk, J, S]))
                    nc.vector.tensor_mul(wc[:, lo:hi], wtf_sb[:, lo:hi],
                                         ce[:])
                else:
                    # small trailing group: fused broadcast, short serial
                    # latency into the next iteration's first matmuls
                    nc.vector.tensor_mul(
                        wc[:, lo:hi], wtf_sb[:, lo:hi],
                        cb[:].unsqueeze(3).broadcast_to([128, gk, J, S]))

            for r in range(NUM_ROUTING):
                last = r == NUM_ROUTING - 1

                # ---- s = X @ (c*W), k-outer so PE consumes wc groups in
                # completion order across the iteration boundary ----
                if not last:
                    for k in range(NKF):
                        rhs = wtf_sb[:, k] if r == 0 else wc[:, k]
                        for g in range(NB):
                            nc.tensor.matmul(
                                sps[g][:],
                                xtf_t[k // 8][:, k % 8,
                                              g * 128:(g + 1) * 128],
                                rhs,
                                start=(k == 0),
                                stop=(k == NKF - 1),
                            )
                    squash_full()
                    if r == 0:
                        # deferred: not needed before iteration 3, keeps
                        # head DMA bandwidth for xtf
                        nc.scalar.dma_start(xtb_sb[:], xtb_d.ap())

                    # ---- M = x^T v + bdelta, pipelined across engines ----
                    for q in range(NQ):
                        x2p = x2rot.tile([128, NB, 8 * 128], bf16,
                                         name="x2p", tag="x2p")
                        nc.sync.dma_start(x2p[:], x2f_d.ap()[:, q])
                        for c3, nkk in ((0, 3), (1, 3), (2, 2)):
                            kabs = q * 8 + c3 * 3
                            mp3 = mpsum.tile([128, nkk, JT], f32,
                                             name="mp3", tag="mp3")
                            for kk in range(nkk):
                                for g in range(NB):
                                    nc.tensor.matmul(
                                        mp3[:, kk, :],
                                        x2p[:, g,
                                            (c3 * 3 + kk) * 128:
                                            (c3 * 3 + kk + 1) * 128],
                                        v_bf[:, g],
                                        start=(g == 0),
                                        stop=(g == NB - 1),
                                    )
                            # tmp = (W*0.1) * M, reduce over t -> tja
                            if c3 == 0 or q == 8:
                                tmpv = tmprot.tile([128, nkk, J, S], bf16,
                                                   name="tmpv", tag="tmpv")
                                nc.vector.tensor_mul(
                                    tmpv[:],
                                    mp3[:].rearrange(
                                        "p c (j t) -> p c j t", t=S),
                                    wtf_sb[:, kabs:kabs + nkk],
                                )
                                nc.vector.reduce_sum(
                                    tja[:, kabs:kabs + nkk], tmpv[:],
                                    axis=AX)
                            else:
                                mcp = tmprot.tile([128, nkk, J, S], bf16,
                                                  name="mcp", tag="mcp")
                                nc.scalar.copy(
                                    mcp[:],
                                    mp3[:].rearrange(
                                        "p c (j t) -> p c j t", t=S))
                                gm = tmprot.tile([128, nkk, J, S], bf16,
                                                 name="gm", tag="gm")
                                nc.gpsimd.tensor_tensor(
                                    gm[:], mcp[:],
                                    wtf_sb[:, kabs:kabs + nkk], op=mult)
                                nc.vector.reduce_sum(
                                    tja[:, kabs:kabs + nkk], gm[:],
                                    axis=AX)
                        if q == 2:
                            chain_group(0, 24)
                        elif q == 5:
                            chain_group(24, 48)
                        elif q == 7:
                            chain_group(48, 64)
                        elif q == 8:
                            chain_group(64, 72)
                            # pull the exp->sqrt act-table swap off the
                            # next squash's critical path
                            dummy = work.tile([128, 1], f32, name="dum",
                                              tag="dum")
                            nc.scalar.sqrt(dummy[:], rep_sb[:, :1])
                else:
                    # last iteration: only this core's 64-row output shard
                    spb = sps[0]
                    for k in range(NKF):
                        nc.tensor.matmul(
                            spb[:BSH, :],
                            xtb_sb[:, k, :],
                            wc[:, k],
                            start=(k == 0),
                            stop=(k == NKF - 1),
                        )
                    sb16 = cold.tile([BSH, J, S], f32, name="sb16",
                                     tag="sb16")
                    nc.vector.tensor_copy(sb16[:], spb[:BSH, :].rearrange(
                        "p (j t) -> p j t", t=S))
                    sqb = cold.tile([BSH, J, S], f32, name="sqb", tag="sqb")
                    nc.vector.tensor_mul(sqb[:], sb16[:], sb16[:])
                    msqb = cold.tile([BSH, J], f32, name="msqb", tag="msqb")
                    nc.vector.reduce_sum(msqb[:], sqb[:], axis=AX)
                    d1b = cold.tile([BSH, J], f32, name="d1b", tag="d1b")
                    nc.vector.tensor_scalar_add(d1b[:], msqb[:], 1.0)
                    rd1b = cold.tile([BSH, J], f32, name="rd1b", tag="rd1b")
                    nc.vector.reciprocal(rd1b[:], d1b[:])
                    magb = cold.tile([BSH, J], f32, name="magb", tag="magb")
                    nc.scalar.sqrt(magb[:], msqb[:])
                    f2b = cold.tile([BSH, J], f32, name="f2b", tag="f2b")
                    nc.vector.tensor_mul(f2b[:], magb[:], rd1b[:])
                    nc.vector.tensor_mul(
                        vout[:].rearrange("p (j t) -> p j t", t=S),
                        sb16[:],
                        f2b[:].unsqueeze(2).broadcast_to([BSH, J, S]),
                    )
                    nc.sync.dma_start(out_d.ap(), vout[:])

    nc.compile()
    return nc


def _get_built():
    global _built
    if _built is None:
        _built = _build()
    return _built


def _prep_inputs(x, weight):
    x = np.asarray(x, dtype=np.float32)
    weight = np.asarray(weight, dtype=np.float32)
    # replicated full tensors, k = c*8 + i, majorized to [partition, ...]
    xtf = np.ascontiguousarray(x.transpose(2, 1, 0)).reshape(KF, B)
    xtf_m = np.ascontiguousarray(
        xtf.reshape(NKF, 128, B).transpose(1, 0, 2)
    ).astype(_BF16)                       # [128, NKF, B]
    x2f = np.ascontiguousarray(x.transpose(0, 2, 1)).reshape(B, KF)
    x2f_m = np.ascontiguousarray(
        x2f.reshape(NB, 128, NQ, 8 * 128).transpose(1, 2, 0, 3)
    ).astype(_BF16)                       # [128, NQ, NB, 1024]
    wtf = np.ascontiguousarray(weight.transpose(0, 3, 1, 2)).reshape(KF, JT)
    wtf_m = np.ascontiguousarray(
        (wtf * 0.1).reshape(NKF, 128, J, S).transpose(1, 0, 2, 3)
    ).astype(_BF16)                       # [128, NKF, J, S]
    xtf_q = np.ascontiguousarray(
        xtf_m.reshape(128, NQ, 8, B))     # [128, NQ, 8, B]
    in_maps = []
    for core in range(NCORES):
        in_maps.append({
            "xtf": xtf_q,
            "wtf": wtf_m,
            "x2f": x2f_m,
            "xtb": np.ascontiguousarray(
                xtf_m[:, :, core * BSH:(core + 1) * BSH]),
        })
    return in_maps


def run(x, weight, trace=False, warmup=1, **kw):
    from concourse import bass_utils
    nc = _get_built()
    in_maps = _prep_inputs(x, weight)
    for _ in range(warmup):
        bass_utils.run_bass_kernel_spmd(
            nc, in_maps, core_ids=list(range(NCORES)), trace=False
        )
    res = bass_utils.run_bass_kernel_spmd(
        nc, in_maps, core_ids=list(range(NCORES)), trace=trace, **kw
    )
    out = np.concatenate(
        [np.asarray(res.results[c]["out"], dtype=np.float32)
         for c in range(NCORES)], axis=0,
    )
    return out.reshape(B, J, S, 1), res


def kernel(x, weight):
    out, _ = run(x, weight)
    return out
